# revision 11
# baseline (speedup 1.0000x reference)
"""BiLSTM-CRF Trainium2 kernel (8-core SPMD).

- Chunked LSTM across 8 cores (4 fwd / 4 bwd chunks); non-initial chunks warm
  up L steps from zero state (forget-gate contraction -> ~ulp-exact h).
- Per-core: on-device input projection, sequential fp32 LSTM recurrence
  (weight-stationary PE matvec), AllGather of hidden sequences, replicated
  output projection, linear-domain (scaled-HMM) Viterbi forward scan on DVE
  (sidesteps the low-precision ACT exp/log tables; exp(feats) via an accurate
  DVE polynomial).
- Host: embedding gather, weight packing, final score + backtrace.
"""

import os
import numpy as np

import concourse.bass as bass
import concourse.mybir as mybir
import concourse.tile as tile
from concourse.bass_utils import run_bass_kernel_spmd
from concourse.vector_clock import ScopedClock

# ---------------------------------------------------------------- patches ---
# This container's walrus build rejects instructions carrying more than one
# semaphore sync-wait. Spread excess waits over same-engine NoOp carriers, and
# split the TileContext exit drain the same way.
_MAXW = 1


def _patched_drain_and_barrier(self, tick_clock, wait_clock):
    nc = self.nc
    carrier = nc.sync.nop(nofuse=True)
    wait_clock.add_sem_waits(carrier.ins, ScopedClock({None: tick_clock.global_clock}))
    si = carrier.ins.sync_info
    waits = list(si.on_wait or [])
    if len(waits) > _MAXW:
        si.on_wait = waits[:_MAXW]
        carrier.ins.sync_info = si
        for i in range(_MAXW, len(waits), _MAXW):
            nop_i = nc.sync.nop(nofuse=True)
            nop_i.ins.sync_info = mybir.SyncInfo(on_wait=waits[i:i + _MAXW], on_update=[])
    nc.sync.drain()
    nc.all_engine_barrier()
    assert self.sems is not None
    popped = nc._tile_sem_poison_stack.pop()
    assert popped is self._sem_poison
    nc.clear_and_free_semaphores(list(self.sems.allocated().values()))
    nc.all_engine_barrier()


tile.TileContext._drain_and_barrier = _patched_drain_and_barrier


def _fix_excess_waits(nc, max_waits=_MAXW):
    nfix = 0
    for fn in nc.m.functions:
        for bb in fn.blocks:
            insts = list(bb.instructions)
            if not any(i.sync_info and i.sync_info.on_wait and len(i.sync_info.on_wait) > max_waits
                       for i in insts):
                continue
            new = []
            for inst in insts:
                si = inst.sync_info
                if si and si.on_wait and len(si.on_wait) > max_waits:
                    waits = list(si.on_wait)
                    chunks = [waits[k:k + max_waits] for k in range(0, len(waits), max_waits)]
                    for ci, ch in enumerate(chunks[:-1]):
                        nop = mybir.InstNoOp(name=f"{inst.name}-wc{ci}", ins=[], outs=[])
                        nop.engine = inst.engine
                        nop.sync_info = mybir.SyncInfo(on_wait=ch, on_update=[])
                        new.append(nop)
                        nfix += 1
                    inst.sync_info = mybir.SyncInfo(on_wait=chunks[-1],
                                                    on_update=list(si.on_update or []))
                new.append(inst)
            bb.instructions = new
    return nfix


def _install_trace_support():
    """Make trace=True work: provide the missing antenv.axon_hooks and disable
    artifact upload (no bucket creds here)."""
    import sys
    import types
    try:
        import antenv.axon_hooks  # noqa: F401
    except ImportError:
        try:
            from trn_agent_boot.trn_boot import _ntff_profile_via_ctypes
            hook = _ntff_profile_via_ctypes('/opt/axon/libaxon_pjrt.so')
            mod = types.ModuleType('antenv.axon_hooks')
            state = {'hook': hook}
            mod.get_axon_ntff_profile_hook = lambda: state['hook']
            mod.set_axon_ntff_profile_hook = lambda h: state.update(hook=h)
            sys.modules['antenv.axon_hooks'] = mod
            import antenv
            antenv.axon_hooks = mod
        except Exception:
            pass
    import concourse.bass_utils as bu
    bu.upload_artifacts = lambda tmpdir: f"local:{tmpdir}"


# ---------------------------------------------------------------- problem ---
T = 4096
EMB = 256
HID = 512
TAGS = 32
START = 30
STOP = 31
NEG = -10000.0

G = 4 * HID
KT = HID // 128        # 4
MT = G // 128          # 16
EKT = EMB // 128       # 2

L = 192
WIN = 1168
CHUNK_STARTS = [0, 1168, 2144, 3120]
CHUNK_SIZES = [1168, 976, 976, 976]

RESCALE_EVERY = 4

F32 = mybir.dt.float32
U32 = mybir.dt.uint32
I32 = mybir.dt.int32
AX = mybir.AxisListType.X
OP = mybir.AluOpType
AF = mybir.ActivationFunctionType

_prog_cache = {}
last_exec_ns = None
last_results = None


def _pexp(nc, pool, out_i32view, x_ap, npart, nfree):
    """exp(x) elementwise via fp32 polynomial on DVE (~1-2 ulp)."""
    LOG2E = 1.4426950408889634
    C1 = 0.693359375
    C2 = -2.12194440e-4
    MAGIC = 12582912.0  # 1.5 * 2**23
    A = [1.0, 1.0, 0.5, 1.6666667163e-1, 4.1665795894e-2, 8.3334519073e-3,
         1.3981999507e-3, 1.9875691500e-4]
    sh = [npart, nfree]
    z = pool.tile(sh, F32, tag="pexp_z", name="pexp_z")
    nc.vector.tensor_scalar(z[:], x_ap, LOG2E, MAGIC, OP.mult, OP.add)
    mf = pool.tile(sh, F32, tag="pexp_mf", name="pexp_mf")
    nc.vector.tensor_scalar(mf[:], z[:], MAGIC, None, OP.subtract)
    mi = pool.tile(sh, I32, tag="pexp_mi", name="pexp_mi")
    nc.vector.tensor_copy(mi[:], mf[:])
    q = pool.tile(sh, F32, tag="pexp_q", name="pexp_q")
    nc.vector.tensor_scalar(q[:], mf[:], C1, None, OP.mult)
    r = pool.tile(sh, F32, tag="pexp_r", name="pexp_r")
    nc.vector.tensor_tensor(r[:], x_ap, q[:], OP.subtract)
    nc.vector.tensor_scalar(q[:], mf[:], C2, None, OP.mult)
    nc.vector.tensor_tensor(r[:], r[:], q[:], OP.subtract)
    w = pool.tile(sh, F32, tag="pexp_w", name="pexp_w")
    nc.vector.tensor_scalar(w[:], r[:], A[7], None, OP.mult)
    for k in range(6, 0, -1):
        nc.vector.scalar_tensor_tensor(w[:], w[:], A[k], r[:], OP.add, OP.mult)
    nc.vector.tensor_scalar(w[:], w[:], A[0], None, OP.add)
    nc.vector.tensor_scalar(mi[:], mi[:], 23, None, OP.logical_shift_left)
    nc.vector.tensor_tensor(out_i32view, w[:].bitcast(I32), mi[:], OP.add)


def _build_program(win_steps, vit_steps):
    key = (win_steps, vit_steps)
    if key in _prog_cache:
        return _prog_cache[key]
    WS = win_steps
    VT = vit_steps

    nc = bass.Bass()
    dp = nc.declare_dram_parameter
    xT = dp("xT", [128, EKT * WS], F32, isOutput=False)
    wihT = dp("wihT", [128, EKT * MT * 128], F32, isOutput=False)
    whhT = dp("whhT", [128, KT * MT * 128], F32, isOutput=False)
    biasg = dp("biasg", [128, MT], F32, isOutput=False)
    h0c0 = dp("h0c0", [128, 8], F32, isOutput=False)
    woutT = dp("woutT", [128, 2 * KT * TAGS], F32, isOutput=False)
    bout = dp("bout", [TAGS, 1], F32, isOutput=False)
    ETm = dp("ETm", [TAGS, TAGS], F32, isOutput=False)
    zinit = dp("zinit", [TAGS, 1], F32, isOutput=False)

    cmb_out = dp("cmb_out", [TAGS, 2 * VT + 16], F32, isOutput=True)
    zfin_out = dp("zfin_out", [TAGS, 1], F32, isOutput=True)

    send_b = nc.dram_tensor("send_b", [128, 2 * 4 * WS], F32)
    gath_b = nc.dram_tensor("gath_b", [8 * 128, 2 * 4 * WS], F32, addr_space="Shared")

    with tile.TileContext(nc) as tc:
        with (
            tc.tile_pool(name="pers", bufs=1) as pers,
        ):
            featsT = pers.tile([TAGS, T], F32)

            with tc.tile_pool(name="mid", bufs=1) as midp:
                hseq = midp.tile([128, 4 * WS], F32)
                hrev = midp.tile([128, 4 * WS], F32)
                Amat = midp.tile([128, MT * WS], F32)
                nc.gpsimd.memset(hseq[:], 0.0)
                nc.gpsimd.memset(hrev[:], 0.0)

                # -------- phase 1: input projection --------
                with tc.tile_pool(name="proj", bufs=1) as projp, \
                     tc.tile_pool(name="projps", bufs=2, space="PSUM") as projpp:
                    xTs = projp.tile([128, EKT * WS], F32)
                    nc.sync.dma_start(xTs[:], xT[:])
                    wihs = projp.tile([128, EKT * MT * 128], F32)
                    nc.sync.dma_start(wihs[:], wihT[:])
                    bs = projp.tile([128, MT], F32)
                    nc.sync.dma_start(bs[:], biasg[:])
                    Av = Amat[:].rearrange("p (t m) -> p t m", m=MT)
                    nblk = (WS + 511) // 512
                    for bi in range(nblk):
                        t0 = bi * 512
                        tn = min(512, WS - t0)
                        for mu in range(MT):
                            aps = projpp.tile([128, 512], F32, tag="aps", name=f"aps{bi}_{mu}")
                            for kk in range(EKT):
                                lhsT = wihs[:, (kk * MT + mu) * 128:(kk * MT + mu) * 128 + 128]
                                rhs = xTs[:, kk * WS + t0: kk * WS + t0 + tn]
                                nc.tensor.matmul(aps[:, 0:tn], lhsT, rhs,
                                                 start=(kk == 0), stop=(kk == EKT - 1))
                            nc.vector.tensor_scalar(Av[:, t0:t0 + tn, mu], aps[:, 0:tn],
                                                    bs[:, mu:mu + 1], None, OP.add)

                # -------- phase 2: recurrence --------
                with tc.tile_pool(name="rec", bufs=1) as recp, \
                     tc.tile_pool(name="recps", bufs=2, space="PSUM") as recpp:
                    whhs = recp.tile([128, KT * MT * 128], F32)
                    nc.sync.dma_start(whhs[:], whhT[:])
                    state = recp.tile([128, 8], F32)
                    nc.sync.dma_start(state[:], h0c0[:])
                    hcur = state[:, 0:4]
                    ccur = state[:, 4:8]

                    def rec_body(iv):
                        ps = recpp.tile([128, MT], F32, tag="recps_t", name="recps_t")
                        for mu in range(MT):
                            for kk in range(KT):
                                lhsT = whhs[:, (kk * MT + mu) * 128:(kk * MT + mu) * 128 + 128]
                                nc.tensor.matmul(ps[:, mu:mu + 1], lhsT, hcur[:, kk:kk + 1],
                                                 start=(kk == 0), stop=(kk == KT - 1))
                        u = recp.tile([128, MT], F32, tag="rec_u", name="rec_u")
                        acol = Amat[:, bass.ds(iv * MT, MT)]
                        nc.vector.tensor_tensor(u[:], ps[:], acol, OP.add)
                        tio = recp.tile([128, 12], F32, tag="rec_tio", name="rec_tio")
                        nc.scalar.activation(tio[:], u[:, 0:12], AF.Tanh, scale=0.5)
                        sg = recp.tile([128, 12], F32, tag="rec_sg", name="rec_sg")
                        nc.vector.tensor_scalar(sg[:], tio[:], 0.5, 0.5, OP.mult, OP.add)
                        gt = recp.tile([128, 4], F32, tag="rec_gt", name="rec_gt")
                        nc.scalar.activation(gt[:], u[:, 12:16], AF.Tanh)
                        p1 = recp.tile([128, 4], F32, tag="rec_p1", name="rec_p1")
                        nc.vector.tensor_tensor(p1[:], sg[:, 0:4], gt[:], OP.mult)
                        p2 = recp.tile([128, 4], F32, tag="rec_p2", name="rec_p2")
                        nc.vector.tensor_tensor(p2[:], sg[:, 4:8], ccur, OP.mult)
                        nc.vector.tensor_tensor(ccur, p1[:], p2[:], OP.add)
                        th = recp.tile([128, 4], F32, tag="rec_th", name="rec_th")
                        nc.scalar.activation(th[:], ccur, AF.Tanh)
                        nc.vector.tensor_tensor(hcur, sg[:, 8:12], th[:], OP.mult)
                        nc.scalar.copy(hseq[:, bass.ds(iv * 4, 4)], hcur)

                    tc.For_i_unrolled(0, WS, 1, rec_body, max_unroll=8)

                    # static block-reversal (dynamic mirrored offsets hit a
                    # lowering bug in this container's toolchain)
                    for j in range(WS):
                        eng = nc.vector.tensor_copy if (j % 2 == 0) else nc.scalar.copy
                        eng(hrev[:, (WS - 1 - j) * 4:(WS - j) * 4], hseq[:, j * 4:j * 4 + 4])

                # -------- phase 3: exchange --------
                nc.sync.dma_start(send_b[:, 0:4 * WS], hseq[:])
                nc.sync.dma_start(send_b[:, 4 * WS:8 * WS], hrev[:])
                nc.gpsimd.collective_compute(
                    "AllGather", OP.bypass,
                    ins=[send_b.ap().opt()], outs=[gath_b.ap().opt()],
                    replica_groups=[list(range(8))],
                )

            # -------- phase 4: assemble + output projection --------
            with tc.tile_pool(name="feat", bufs=1) as fp, \
                 tc.tile_pool(name="featps", bufs=2, space="PSUM") as fpp:
                hfT = fp.tile([128, 4 * T], F32)
                hbT = fp.tile([128, 4 * T], F32)
                for c in range(4):
                    t0 = CHUNK_STARTS[c]
                    sz = CHUNK_SIZES[c]
                    skip = WS - sz
                    nc.sync.dma_start(hfT[:, 4 * t0: 4 * (t0 + sz)],
                                      gath_b[128 * c:128 * (c + 1), 4 * skip: 4 * WS])
                    tb0 = T - (t0 + sz)
                    nc.sync.dma_start(hbT[:, 4 * tb0: 4 * (tb0 + sz)],
                                      gath_b[128 * (4 + c):128 * (5 + c), 4 * WS: 4 * WS + 4 * sz])
                wouts = fp.tile([128, 2 * KT * TAGS], F32)
                nc.sync.dma_start(wouts[:], woutT[:])
                bouts = fp.tile([TAGS, 1], F32)
                nc.sync.dma_start(bouts[:], bout[:])
                hfv = hfT[:].rearrange("p (t k) -> p t k", k=4)
                hbv = hbT[:].rearrange("p (t k) -> p t k", k=4)
                for bi in range(T // 512):
                    t0 = bi * 512
                    fps = fpp.tile([TAGS, 512], F32, tag="fps", name=f"fps{bi}")
                    for kt in range(2 * KT):
                        lhsT = wouts[:, kt * TAGS:(kt + 1) * TAGS]
                        kk = kt % KT
                        rhs = (hfv if kt < KT else hbv)[:, t0:t0 + 512, kk]
                        nc.tensor.matmul(fps[:], lhsT, rhs,
                                         start=(kt == 0), stop=(kt == 2 * KT - 1))
                    nc.vector.tensor_scalar(featsT[:, t0:t0 + 512], fps[:],
                                            bouts[:], None, OP.add)

            # -------- phase 5: linear-domain viterbi --------
            with tc.tile_pool(name="vit", bufs=1) as vp:
                D = vp.tile([TAGS, T], F32)
                _pexp(nc, vp, D[:].bitcast(I32), featsT[:], TAGS, T)
                ETs = vp.tile([TAGS, TAGS], F32)
                nc.sync.dma_start(ETs[:], ETm[:])
                zin = vp.tile([TAGS, 1], F32)
                nc.sync.dma_start(zin[:], zinit[:])
                cmb = vp.tile([TAGS, 2 * VT + 16], F32)
                nc.gpsimd.memset(cmb[:], 0.0)

                zblk = vp.tile([TAGS, TAGS], F32)
                nc.gpsimd.memset(zblk[:], 0.0)
                nc.vector.tensor_tensor(zblk[:, 0:1], zin[:], D[:, 0:1], OP.mult)

                zt = vp.tile([TAGS, TAGS], F32)
                zb = vp.tile([TAGS, TAGS], F32)
                Mt = vp.tile([TAGS, TAGS], F32)
                mx8 = vp.tile([TAGS, 8], F32)
                ix8 = vp.tile([TAGS, 8], U32)
                vsum = vp.tile([TAGS, 1], F32)
                rmz = vp.tile([TAGS, 1], F32)
                dcols = vp.tile([TAGS, 8], F32)
                blk = vp.tile([TAGS, 16], F32)

                # per group of <=8 steps: one dynamic DVE read (D columns), one
                # dynamic ACT write (bp/mz block) -- this container's lowering
                # supports only one dynamic-offset expression per engine/loop.
                def step(i, bp_dst, mz_dst, dcol, rescale):
                    nc.vector.transpose(zt[:], zblk[:])
                    nc.vector.stream_shuffle(zb[:], zt[:], [0] * 32)
                    if rescale:
                        nc.vector.tensor_reduce(mz_dst, zb[:], AX, OP.max)
                        nc.vector.reciprocal(rmz[:], mz_dst)
                        nc.vector.scalar_tensor_tensor(Mt[:], zb[:], rmz[:], ETs[:],
                                                       OP.mult, OP.mult)
                    else:
                        nc.vector.tensor_tensor(Mt[:], zb[:], ETs[:], OP.mult)
                    nc.vector.max_with_indices(mx8[:], ix8[:], Mt[:])
                    nc.scalar.copy(bp_dst, ix8[:, 0:1])
                    nc.vector.tensor_reduce(vsum[:], Mt[:], AX, OP.add)
                    nc.vector.tensor_tensor(zblk[:, 0:1], vsum[:], dcol, OP.mult)

                # steps 1..7 statically, results into the pad region [2*VT, 2*VT+16)
                for t in range(1, 8):
                    step(t, cmb[:, 2 * VT + t:2 * VT + t + 1],
                         cmb[:, 2 * VT + 8 + t:2 * VT + 9 + t],
                         D[:, t:t + 1], rescale=(t % RESCALE_EVERY == 1))

                def vit_body(iv0, unroll):
                    nc.vector.tensor_copy(dcols[:, 0:unroll], D[:, bass.ds(iv0, unroll)])
                    for i in range(unroll):
                        step(None, blk[:, i:i + 1], blk[:, 8 + i:9 + i],
                             dcols[:, i:i + 1], rescale=(i % RESCALE_EVERY == 0))
                    nc.scalar.copy(cmb[:, bass.ds(iv0 * 2, 16)], blk[:])

                tc.For_i_unrolled_general(8, VT, 1, vit_body, max_unroll=8)

                nc.sync.dma_start(cmb_out[:], cmb[:])
                nc.sync.dma_start(zfin_out[:], zblk[:, 0:1])

    _fix_excess_waits(nc)
    _prog_cache[key] = nc
    return nc


def _pack_weights(W_ih, W_hh, b):
    """Gate-row order [i,f,g,o] -> [i,f,o,g]; build (k,m) lhsT tiles."""
    perm = np.concatenate([np.arange(0, 1024), np.arange(1536, 2048),
                           np.arange(1024, 1536)])
    Wih = W_ih[perm]
    Whh = W_hh[perm]
    bp = b[perm]
    wih_t = np.zeros((128, EKT * MT * 128), np.float32)
    for kk in range(EKT):
        for mu in range(MT):
            blk = Wih[mu * 128:(mu + 1) * 128, kk * 128:(kk + 1) * 128]
            wih_t[:, (kk * MT + mu) * 128:(kk * MT + mu) * 128 + 128] = blk.T
    whh_t = np.zeros((128, KT * MT * 128), np.float32)
    for kk in range(KT):
        for mu in range(MT):
            blk = Whh[mu * 128:(mu + 1) * 128, kk * 128:(kk + 1) * 128]
            whh_t[:, (kk * MT + mu) * 128:(kk * MT + mu) * 128 + 128] = blk.T
    bg = np.zeros((128, MT), np.float32)
    for mu in range(MT):
        bg[:, mu] = bp[mu * 128:(mu + 1) * 128]
    return wih_t, whh_t, bg


def kernel(tokens, emb, W_ih_f, W_hh_f, b_f, h0_f, c0_f,
           W_ih_b, W_hh_b, b_b, h0_b, c0_b, W_out, b_out, trans):
    global last_exec_ns, last_results
    tokens = np.asarray(tokens)
    emb = np.asarray(emb, np.float32)
    x = emb[tokens.astype(np.int64)]
    xr = x[::-1].copy()

    trace = bool(int(os.environ.get("KERNEL_TRACE", "0")))
    if trace:
        _install_trace_support()

    nc = _build_program(WIN, T)

    wih_f, whh_f, bg_f = _pack_weights(np.asarray(W_ih_f, np.float32),
                                       np.asarray(W_hh_f, np.float32),
                                       np.asarray(b_f, np.float32))
    wih_b, whh_b, bg_b = _pack_weights(np.asarray(W_ih_b, np.float32),
                                       np.asarray(W_hh_b, np.float32),
                                       np.asarray(b_b, np.float32))

    def state_pack(h0, c0, zero):
        s = np.zeros((128, 8), np.float32)
        if not zero:
            for kk in range(4):
                s[:, kk] = h0[kk * 128:(kk + 1) * 128]
                s[:, 4 + kk] = c0[kk * 128:(kk + 1) * 128]
        return s

    W_out = np.asarray(W_out, np.float32)
    wout_t = np.zeros((128, 2 * KT * TAGS), np.float32)
    for kt in range(2 * KT):
        wout_t[:, kt * TAGS:(kt + 1) * TAGS] = W_out[:, kt * 128:(kt + 1) * 128].T
    bout_v = np.asarray(b_out, np.float32).reshape(TAGS, 1)

    trans = np.asarray(trans, np.float32)
    ET = np.exp(trans.astype(np.float64)).astype(np.float32).T.copy()  # [n,p]

    alpha0 = np.full(TAGS, NEG, np.float32)
    alpha0[START] = 0.0
    smat0 = alpha0[:, None] + trans
    m0 = smat0.max(axis=0)
    lse0 = (m0 + np.log(np.exp(smat0 - m0).sum(axis=0))).astype(np.float32)
    C0 = float(lse0.max())
    zinit_v = np.exp((lse0 - C0).astype(np.float32)).reshape(TAGS, 1).astype(np.float32)

    def xwin(xarr, c):
        t0 = CHUNK_STARTS[c]
        s = 0 if c == 0 else t0 - L
        w = xarr[s:s + WIN]
        out = np.zeros((128, EKT * WIN), np.float32)
        for kk in range(EKT):
            out[:, kk * WIN:(kk + 1) * WIN] = w[:, kk * 128:(kk + 1) * 128].T
        return out

    in_maps = []
    for core in range(8):
        fwd = core < 4
        c = core % 4
        in_maps.append({
            "xT": xwin(x if fwd else xr, c),
            "wihT": wih_f if fwd else wih_b,
            "whhT": whh_f if fwd else whh_b,
            "biasg": bg_f if fwd else bg_b,
            "h0c0": state_pack(np.asarray(h0_f if fwd else h0_b, np.float32),
                               np.asarray(c0_f if fwd else c0_b, np.float32),
                               zero=(c != 0)),
            "woutT": wout_t,
            "bout": bout_v,
            "ETm": np.ascontiguousarray(ET),
            "zinit": zinit_v,
        })

    res = run_bass_kernel_spmd(nc, in_maps, core_ids=list(range(8)), trace=trace)
    last_results = res
    last_exec_ns = res.exec_time_ns
    r0 = res.results[0]

    cmb = r0["cmb_out"].astype(np.float64)
    zf = r0["zfin_out"].astype(np.float64).reshape(TAGS)

    bps = np.zeros((TAGS, T), np.int64)
    logmz = 0.0
    for t in range(1, 8):
        bps[:, t] = cmb[:, 2 * T + t].astype(np.int64)
        if t % RESCALE_EVERY == 1:
            logmz += np.log(cmb[0, 2 * T + 8 + t])
    for g in range(8, T, 8):
        blkg = cmb[:, 2 * g:2 * g + 16]
        for i in range(8):
            bps[:, g + i] = blkg[:, i].astype(np.int64)
            if i % RESCALE_EVERY == 0:
                logmz += np.log(blkg[0, 8 + i])

    Ctot = C0 + logmz
    with np.errstate(divide="ignore"):
        alpha_T = np.log(zf) + Ctot
    final = alpha_T + trans[:, STOP].astype(np.float64)
    mf = final.max()
    score = np.float32(mf + np.log(np.exp(final - mf).sum()))
    best = int(np.argmax(final))

    path = np.empty(T, np.int64)
    path[T - 1] = best
    cur = best
    for t in range(T - 1, 0, -1):
        cur = int(bps[cur, t])
        path[t - 1] = cur
    return np.float32(score), path.astype(np.int32)
